# revision 2
# baseline (speedup 1.0000x reference)
"""Causal self-attention (GPT-2 block) for Trainium2, 8 NeuronCores.

v5 = v3 + bf16 x path (host casts x to bf16: half the x DMA, 1 cyc/col
PE transposes) + first-two-chunk xt8 casts on Vector instead of GpSimd
(removes the startup stalls waiting on slow GpSimd casts).

v3 over the 473us baseline:
 - Q/K GEMMs in fp8e4 DoubleRow (2 contraction blocks per pass -> half
   the streamed columns; ~2x on hardware). Weights pre-scaled x32 on the
   host to clear the fp8e4 subnormal floor; 1/32 + bias folded into the
   PSUM->SBUF copy. x reaches the QK GEMM via an fp8 xt copy written by
   the otherwise-idle GpSimd engine.
 - V and proj GEMMs in bf16 (same PE rate as f32r; fp8 there costs 3e-2
   rel err because V/attn-out quantization hits the output linearly,
   measured in a host sim, while QK-fp8 only costs 8.6e-3 through the
   softmax).
 - Attention is software-pipelined: scores block i+1 issues before PV
   block i, so the PE never waits on the Scalar exp (the old kernel's
   tail ran scores->exp->PV serially, inflating 0.9ns/col matmuls to
   1.4-1.55; the HAM k=4 records are a symptom of those stalls, not a
   clock throttle - a pure attention-shaped stream microbenches at full
   rate).

Sharding: core = 2*batch + head_group (one batch + 8 of 16 heads,
Megatron split); V-bias/proj-bias folded into a host-side additive
correction; the two head-group partial proj outputs per batch summed on
the host.
"""

import numpy as np

import concourse.bass as bass
import concourse.tile as tile
from concourse import bacc, mybir
from concourse.bass_utils import run_bass_kernel_spmd
from concourse.masks import make_identity, make_lower_triangular

# Problem shape (fixed by the harness contract).
B, S, D, H, HD = 4, 2048, 1024, 16, 64
NCORES = 8
HG = 8                # heads per core
FG = HG * HD          # 512 features per head group
P = 128
DB = D // P           # 8 contraction blocks
FBN = FG // P         # 4 feature blocks
SC = 512              # attention sequence chunk
NQ = S // SC          # 4
NKB = S // P          # 16 key blocks
F32 = mybir.dt.float32
F32R = mybir.dt.float32r
BF16 = mybir.dt.bfloat16
FP8 = mybir.dt.float8e4
DT_MM = F32R
DR = mybir.MatmulPerfMode.DoubleRow
EXP = mybir.ActivationFunctionType.Exp
MULT = mybir.AluOpType.mult
ADD = mybir.AluOpType.add
SCALE = 1.0 / float(HD) ** 0.5
MASKVAL = -1e30
WSCALE = 32.0         # host-side fp8 weight pre-scale (Q/K only)
WINV = 1.0 / WSCALE


class _Ctx:
    """Tiles/pools shared by the emission thunks."""


def _attention_pair_thunks(nc, cx, hA, hB, q):
    """Thunks for one q-chunk of attention for a head pair, software
    pipelined: thunk S_i does scores+mask+exp for block i, thunk P_i the
    PV matmuls; emitted S0 S1 P0 S2 P1 ... Sn P(n-1) Pn so the PE
    streams scores while the Scalar exp for the previous block runs."""
    blocks = [(kb, None) for kb in range(4 * q)] + \
             [(4 * q + jj, jj) for jj in range(4)]
    nblk = len(blocks)
    st = {"sx": {}}

    def setup():
        st["heads"] = []
        for h in (hA, hB):
            out_ps = cx.psout.tile([65, SC], F32, tag="outps")
            st["heads"].append((h, (h % 2) * 64, h // 2, out_ps))

    def make_scores(i, kb, jj):
        def run():
            heads = st["heads"]
            off = 0 if jj is None else jj * P
            w = SC - off
            sts = []
            for h, pb, j, out_ps in heads:
                stp = cx.psst.tile([P, SC], F32, tag="stps")
                nc.tensor.matmul(
                    stp[:, :w],
                    cx.KT[pb:pb + 64, j, kb * P:(kb + 1) * P],
                    cx.QTc[q][pb:pb + 64, j, off:SC],
                    start=True, stop=True, tile_position=(pb, 0))
                if jj is not None:
                    nc.vector.tensor_add(stp[:, :P], stp[:, :P], cx.addmask)
                sts.append(stp)
            sxs = []
            for (h, pb, j, out_ps), stp in zip(heads, sts):
                sx = cx.sxp.tile([P, SC], DT_MM, tag="sx")
                nc.scalar.activation(sx[:, :w], stp[:, :w], EXP, scale=SCALE)
                sxs.append(sx)
            st["sx"][i] = sxs
        return run

    def make_pv(i, kb, jj):
        def run():
            off = 0 if jj is None else jj * P
            w = SC - off
            sxs = st["sx"].pop(i)
            for (h, pb, j, out_ps), sx in zip(st["heads"], sxs):
                nc.tensor.matmul(
                    out_ps[:, off:], cx.V[:, kb, h, :], sx[:, :w],
                    start=(i == 0), stop=(i == nblk - 1))
        return run

    def drain():
        st["raws"] = []
        for h, pb, j, out_ps in st["heads"]:
            raw = cx.nrmraw.tile([65, SC], F32, tag="raw")
            nc.vector.tensor_copy(raw, out_ps)
            st["raws"].append(raw)

    def norm():
        for (h, pb, j, out_ps), raw in zip(st["heads"], st["raws"]):
            # Single-partition reciprocal blocks the DVE FIFO for ~us;
            # DMA-scatter the sums across 128 partitions first.
            rsh = cx.nrmbc.tile([P, SC // P], F32, tag="rsh")
            nc.sync.dma_start(rsh, raw[64:65, :])
            nc.vector.reciprocal(rsh, rsh)
            rdram = cx.drp.tile([1, SC], F32, tag="rdram")
            nc.sync.dma_start(rdram, rsh)
            rb = cx.nrmbc.tile([64, SC], F32, tag="rb")
            nc.sync.dma_start(rb, rdram.to_broadcast([64, SC]))
            stg = cx.nrmbc.tile([64, SC], BF16, tag="stg")
            nc.vector.tensor_mul(stg, raw[0:64, :], rb)
            nc.sync.dma_start(cx.attnTc[q][pb:pb + 64, j, :], stg)

    scores_t = [make_scores(i, kb, jj) for i, (kb, jj) in enumerate(blocks)]
    pv_t = [make_pv(i, kb, jj) for i, (kb, jj) in enumerate(blocks)]
    thunks = [setup, scores_t[0]]
    for i in range(1, nblk):
        thunks.append(scores_t[i])
        thunks.append(pv_t[i - 1])
    thunks.append(pv_t[nblk - 1])
    thunks += [drain, norm]
    return thunks


def _attention_chunk_thunks(nc, cx, q):
    out = []
    for hp in range(HG // 2):
        out += _attention_pair_thunks(nc, cx, 2 * hp, 2 * hp + 1, q)
    return out


def _proj_chunk_thunks(nc, cx, q, out_d):
    """Proj for the s-blocks of chunk q (bf16); two thunks per s-block."""
    thunks = []
    for sb in range(SC // P):
        sblk = q * (SC // P) + sb

        def make_half(hf, sblk=sblk, sb=sb):
            def run():
                og = cx.ogp.tile([P, D // 2], F32, tag="og")
                ps = cx.ps1.tile([P, D // 2], F32, tag="qkps")
                n0 = hf * (D // 2)
                for j in range(FBN):
                    nc.tensor.matmul(
                        ps,
                        cx.attnTc[q][:, j, sb * P:(sb + 1) * P],
                        cx.wp_sb[:, j, n0:n0 + D // 2],
                        start=(j == 0), stop=(j == FBN - 1))
                nc.any.tensor_copy(og, ps)
                nc.sync.dma_start(
                    out_d.ap()[sblk * P:(sblk + 1) * P, n0:n0 + D // 2], og)
            return run

        thunks.append(make_half(0))
        thunks.append(make_half(1))
    return thunks


def _body(tc, x_d, wq_d, wk_d, wv_d, wp_d, bq_d, bk_d, out_d):
    nc = tc.nc
    cx = _Ctx()
    XC = 256                  # QKV s-chunk width
    NXC = S // XC             # 8
    with (
        tc.tile_pool(name="persist", bufs=1) as persist,
        tc.tile_pool(name="ph1", bufs=1) as ph1,
        tc.tile_pool(name="xin", bufs=3) as xinp,
        tc.tile_pool(name="xtp", bufs=2) as xtp,
        tc.tile_pool(name="xtp8", bufs=2) as xtp8,
        tc.tile_pool(name="qtc", bufs=2) as qtc,
        tc.tile_pool(name="atc", bufs=2) as atc,
        tc.tile_pool(name="sxp", bufs=4) as sxp,
        tc.tile_pool(name="nrmraw", bufs=3) as nrmraw,
        tc.tile_pool(name="nrmbc", bufs=2) as nrmbc,
        tc.tile_pool(name="ogp", bufs=2) as ogp,
        # PSUM banks: qkps 2 + (pt+stps shared) 4 + outps 2 = 8
        tc.tile_pool(name="ps1", bufs=2, space="PSUM") as ps1,
        tc.tile_pool(name="psst", bufs=4, space="PSUM") as psst,
        tc.tile_pool(name="psout", bufs=2, space="PSUM") as psout,
        tc.tile_pool(name="drp", bufs=8, space="DRAM") as drp,
    ):
        cx.sxp, cx.nrmraw, cx.nrmbc, cx.ogp = sxp, nrmraw, nrmbc, ogp
        cx.psst, cx.psout, cx.drp, cx.ps1 = psst, psout, drp, ps1

        ident = persist.tile([P, P], F32)
        make_identity(nc, ident)
        ident16 = persist.tile([P, P], BF16)
        nc.vector.tensor_copy(ident16, ident)
        for _ in range(12):
            wp_ps = ps1.tile([P, P], F32, tag="qkps")
            nc.tensor.matmul(wp_ps, ident, ident, start=True, stop=True)
        cx.addmask = persist.tile([P, P], F32)
        make_lower_triangular(nc, cx.addmask, val=MASKVAL, diag=False)
        bq_sb = persist.tile([P, FBN], F32)
        bk_sb = persist.tile([P, FBN], F32)
        nc.sync.dma_start(bq_sb, bq_d.ap().rearrange("(j p) -> p j", p=P))
        nc.sync.dma_start(bk_sb, bk_d.ap().rearrange("(j p) -> p j", p=P))

        cx.KT = persist.tile([P, FBN, S], DT_MM)
        cx.V = persist.tile([P, NKB, HG, HD + 1], DT_MM)
        ones_col = persist.tile([P, 1], F32)
        nc.vector.memset(ones_col, 1.0)
        nc.vector.tensor_copy(cx.V[:, :, :, HD],
                              ones_col.to_broadcast([P, NKB, HG]))
        cx.wp_sb = persist.tile([P, FBN, D], BF16)
        cx.QTc = [qtc.tile([P, FBN, SC], DT_MM, tag="qtc", name=f"qtc{q}")
                  for q in range(NQ)]
        cx.attnTc = [atc.tile([P, FBN, SC], BF16, tag="atc",
                              name=f"atc{q}") for q in range(NQ)]

        wq_sb = ph1.tile([P, DB, FG], FP8)
        wk_sb = ph1.tile([P, DB, FG], FP8)
        wv_sb = ph1.tile([P, DB, FG], BF16)

        def transpose_chunk(xc, xt, xt8):
            thunks = []
            for sb in range(XC // P):
                s0 = xc * XC + sb * P
                for dh in range(2):
                    xin = xinp.tile([P, D // 2], BF16, tag="xin")
                    nc.sync.dma_start(
                        xin, x_d.ap()[s0:s0 + P,
                                      dh * (D // 2):(dh + 1) * (D // 2)])
                    if xc == 0:
                        # Paced pre-warm: junk full-array matmuls keyed to
                        # the arriving input DMAs keep the PE busy through
                        # the initial load window.
                        wp_ps = ps1.tile([P, P], F32, tag="qkps")
                        nc.tensor.matmul(wp_ps, ident16, xin[:, 0:P],
                                         start=True, stop=True)
                    for db4 in range(DB // 2):
                        db = dh * (DB // 2) + db4
                        def t(sb=sb, db=db, db4=db4, xin=xin, xc=xc):
                            pt = cx.psst.tile([P, P], BF16, tag="stps")
                            nc.tensor.transpose(
                                pt, xin[:, db4 * P:(db4 + 1) * P], ident16)
                            nc.vector.tensor_copy(
                                xt[:, db, sb * P:(sb + 1) * P], pt)
                            eng = nc.vector if xc < 2 else nc.gpsimd
                            eng.tensor_copy(
                                xt8[:, db, sb * P:(sb + 1) * P],
                                xt[:, db, sb * P:(sb + 1) * P])
                        thunks.append(t)
            return thunks

        xts = [xtp.tile([P, DB, XC], BF16, tag="xt", name=f"xt{xc}")
               for xc in range(NXC)]
        xt8s = [xtp8.tile([P, DB, XC], FP8, tag="xt8", name=f"xt8{xc}")
                for xc in range(NXC)]

        bg = []          # attention/proj thunks dripped between QKV groups
        tr = []          # transpose thunks for the next chunk

        def drip(ntr, nbg):
            for _ in range(ntr):
                if tr:
                    tr.pop(0)()
            for _ in range(nbg):
                if bg:
                    bg.pop(0)()

        for t in transpose_chunk(0, xts[0], xt8s[0]):
            t()
        for w_sb, w_d in ((wq_sb, wq_d), (wk_sb, wk_d), (wv_sb, wv_d)):
            wr = w_d.ap().rearrange("(db p) f -> db p f", p=P)
            for db in range(DB):
                nc.sync.dma_start(w_sb[:, db], wr[db])
        nc.sync.dma_start(
            cx.wp_sb, wp_d.ap().rearrange("(j p) n -> p j n", p=P))

        for xc in range(NXC):
            xt, xt8 = xts[xc], xt8s[xc]
            q, half = divmod(xc, 2)
            if xc + 1 < NXC:
                tr += transpose_chunk(xc + 1, xts[xc + 1], xt8s[xc + 1])
            if half == 0:
                if q >= 1:
                    bg += _attention_chunk_thunks(nc, cx, q - 1)
                if q >= 2:
                    bg += _proj_chunk_thunks(nc, cx, q - 2, out_d)
            per = (len(bg) + 9) // 10

            # Q and K -> transposed feature-major layout via fp8
            # DoubleRow; 1/WSCALE and bias folded into the copy out.
            for w_sb, Tc, b_sb in ((wq_sb, cx.QTc, bq_sb),
                                   (wk_sb, None, bk_sb)):
                for fb in range(FBN):
                    ps = ps1.tile([P, XC], F32, tag="qkps")
                    for j in range(DB // 2):
                        nc.tensor.matmul(
                            ps,
                            w_sb[:, 2 * j:2 * j + 2,
                                 fb * P:(fb + 1) * P],
                            xt8[:, 2 * j:2 * j + 2, :],
                            start=(j == 0), stop=(j == DB // 2 - 1),
                            perf_mode=DR)
                        drip(1, 0)
                    if Tc is not None:
                        dst = Tc[q][:, fb, half * XC:(half + 1) * XC]
                    else:
                        dst = cx.KT[:, fb, xc * XC:(xc + 1) * XC]
                    nc.vector.tensor_scalar(dst, ps, WINV,
                                            b_sb[:, fb:fb + 1], MULT, ADD)
                    drip(0, per)
            # V -> natural [s, feat] layout, bf16 (no bias: host-folded).
            for sb in range(XC // P):
                kb = xc * (XC // P) + sb
                ps = ps1.tile([P, FG], F32, tag="qkps")
                for db in range(DB):
                    nc.tensor.matmul(
                        ps,
                        xt[:, db, sb * P:(sb + 1) * P],
                        wv_sb[:, db, :],
                        start=(db == 0), stop=(db == DB - 1))
                    drip(1 if db % 2 else 0, 0)
                nc.vector.tensor_copy(
                    cx.V[:, kb, :, 0:HD],
                    ps.rearrange("p (h c) -> p h c", h=HG))
                drip(0, per)
            while tr:
                tr.pop(0)()

        # Tail: attention(3) interleaved with proj(2), then proj(3).
        tail_att = _attention_chunk_thunks(nc, cx, NQ - 1)
        tail_proj = _proj_chunk_thunks(nc, cx, NQ - 2, out_d)
        k = max(1, len(tail_att) // max(1, len(tail_proj)))
        while tail_att or tail_proj:
            for _ in range(k):
                if tail_att:
                    tail_att.pop(0)()
            if tail_proj:
                tail_proj.pop(0)()
        while bg:
            bg.pop(0)()
        for t in _proj_chunk_thunks(nc, cx, NQ - 1, out_d):
            t()


def build_nc():
    nc = bacc.Bacc("TRN2", target_bir_lowering=False)
    x_d = nc.dram_tensor("x", [S, D], BF16, kind="ExternalInput")
    wq_d = nc.dram_tensor("wq", [D, FG], FP8, kind="ExternalInput")
    wk_d = nc.dram_tensor("wk", [D, FG], FP8, kind="ExternalInput")
    wv_d = nc.dram_tensor("wv", [D, FG], BF16, kind="ExternalInput")
    wp_d = nc.dram_tensor("wp", [FG, D], BF16, kind="ExternalInput")
    bq_d = nc.dram_tensor("bq", [FG], F32, kind="ExternalInput")
    bk_d = nc.dram_tensor("bk", [FG], F32, kind="ExternalInput")
    out_d = nc.dram_tensor("out", [S, D], F32, kind="ExternalOutput")
    with tile.TileContext(nc) as tc:
        _body(tc, x_d, wq_d, wk_d, wv_d, wp_d, bq_d, bk_d, out_d)
    nc.compile()
    return nc


_NC = None


def _get_nc():
    global _NC
    if _NC is None:
        _NC = build_nc()
    return _NC


def make_in_maps(hs, w, bvec, pw):
    import ml_dtypes
    fp8 = ml_dtypes.float8_e4m3
    bf16 = ml_dtypes.bfloat16
    in_maps = []
    for core in range(NCORES):
        b, g = divmod(core, 2)
        lo, hi = g * FG, (g + 1) * FG
        in_maps.append({
            "x": np.ascontiguousarray(hs[b]).astype(bf16),
            "wq": np.ascontiguousarray(
                w[:, lo:hi] * WSCALE).astype(fp8),
            "wk": np.ascontiguousarray(
                w[:, D + lo:D + hi] * WSCALE).astype(fp8),
            "wv": np.ascontiguousarray(
                w[:, 2 * D + lo:2 * D + hi]).astype(bf16),
            "wp": np.ascontiguousarray(pw[lo:hi, :]).astype(bf16),
            "bq": np.ascontiguousarray(bvec[lo:hi]),
            "bk": np.ascontiguousarray(bvec[D + lo:D + hi]),
        })
    return in_maps


def combine(parts, bvec, pw, pb):
    bv = bvec[2 * D:3 * D].astype(np.float64)
    corr = (bv @ pw.astype(np.float64) + pb.astype(np.float64)).astype(
        np.float32)
    out = np.empty((B, S, D), np.float32)
    for b in range(B):
        out[b] = parts[2 * b] + parts[2 * b + 1] + corr
    return out


def kernel(hidden_states, c_attn_w, c_attn_b, c_proj_w, c_proj_b,
           **run_kwargs):
    hs = np.asarray(hidden_states, dtype=np.float32)
    w = np.asarray(c_attn_w, dtype=np.float32)
    bvec = np.asarray(c_attn_b, dtype=np.float32)
    pw = np.asarray(c_proj_w, dtype=np.float32)
    pb = np.asarray(c_proj_b, dtype=np.float32)
    nc = _get_nc()
    res = run_bass_kernel_spmd(nc, make_in_maps(hs, w, bvec, pw),
                               core_ids=list(range(NCORES)), **run_kwargs)
    parts = [res.results[i]["out"] for i in range(NCORES)]
    out = combine(parts, bvec, pw, pb)
    if run_kwargs:
        return out, res
    return out


# revision 3
# speedup vs baseline: 1.0272x; 1.0272x over previous
"""Causal self-attention (GPT-2 block) for Trainium2, 8 NeuronCores.

v5 = v3 + bf16 x path (host casts x to bf16: half the x DMA, 1 cyc/col
PE transposes) + first-two-chunk xt8 casts on Vector instead of GpSimd
(removes the startup stalls waiting on slow GpSimd casts).

v3 over the 473us baseline:
 - Q/K GEMMs in fp8e4 DoubleRow (2 contraction blocks per pass -> half
   the streamed columns; ~2x on hardware). Weights pre-scaled x32 on the
   host to clear the fp8e4 subnormal floor; 1/32 + bias folded into the
   PSUM->SBUF copy. x reaches the QK GEMM via an fp8 xt copy written by
   the otherwise-idle GpSimd engine.
 - V and proj GEMMs in bf16 (same PE rate as f32r; fp8 there costs 3e-2
   rel err because V/attn-out quantization hits the output linearly,
   measured in a host sim, while QK-fp8 only costs 8.6e-3 through the
   softmax).
 - Attention is software-pipelined: scores block i+1 issues before PV
   block i, so the PE never waits on the Scalar exp (the old kernel's
   tail ran scores->exp->PV serially, inflating 0.9ns/col matmuls to
   1.4-1.55; the HAM k=4 records are a symptom of those stalls, not a
   clock throttle - a pure attention-shaped stream microbenches at full
   rate).

Sharding: core = 2*batch + head_group (one batch + 8 of 16 heads,
Megatron split); V-bias/proj-bias folded into a host-side additive
correction; the two head-group partial proj outputs per batch summed on
the host.
"""

import numpy as np

import concourse.bass as bass
import concourse.tile as tile
from concourse import bacc, mybir
from concourse.bass_utils import run_bass_kernel_spmd
from concourse.masks import make_identity, make_lower_triangular

# Problem shape (fixed by the harness contract).
B, S, D, H, HD = 4, 2048, 1024, 16, 64
NCORES = 8
HG = 8                # heads per core
FG = HG * HD          # 512 features per head group
P = 128
DB = D // P           # 8 contraction blocks
FBN = FG // P         # 4 feature blocks
SC = 512              # attention sequence chunk
NQ = S // SC          # 4
NKB = S // P          # 16 key blocks
F32 = mybir.dt.float32
F32R = mybir.dt.float32r
BF16 = mybir.dt.bfloat16
FP8 = mybir.dt.float8e4
DT_MM = F32R
DR = mybir.MatmulPerfMode.DoubleRow
EXP = mybir.ActivationFunctionType.Exp
MULT = mybir.AluOpType.mult
ADD = mybir.AluOpType.add
SCALE = 1.0 / float(HD) ** 0.5
MASKVAL = -1e30
WSCALE = 32.0         # host-side fp8 weight pre-scale (Q/K only)
WINV = 1.0 / WSCALE


class _Ctx:
    """Tiles/pools shared by the emission thunks."""


def _attention_pair_thunks(nc, cx, hA, hB, q):
    """Thunks for one q-chunk of attention for a head pair, software
    pipelined: thunk S_i does scores+mask+exp for block i, thunk P_i the
    PV matmuls; emitted S0 S1 P0 S2 P1 ... Sn P(n-1) Pn so the PE
    streams scores while the Scalar exp for the previous block runs."""
    blocks = [(kb, None) for kb in range(4 * q)] + \
             [(4 * q + jj, jj) for jj in range(4)]
    nblk = len(blocks)
    st = {"sx": {}}

    def setup():
        st["heads"] = []
        for h in (hA, hB):
            out_ps = cx.psout.tile([65, SC], F32, tag="outps")
            st["heads"].append((h, (h % 2) * 64, h // 2, out_ps))

    def make_scores(i, kb, jj):
        def run():
            heads = st["heads"]
            off = 0 if jj is None else jj * P
            w = SC - off
            sts = []
            for h, pb, j, out_ps in heads:
                stp = cx.psst.tile([P, SC], F32, tag="stps")
                nc.tensor.matmul(
                    stp[:, :w],
                    cx.KT[pb:pb + 64, j, kb * P:(kb + 1) * P],
                    cx.QTc[q][pb:pb + 64, j, off:SC],
                    start=True, stop=True, tile_position=(pb, 0))
                if jj is not None:
                    nc.vector.tensor_add(stp[:, :P], stp[:, :P], cx.addmask)
                sts.append(stp)
            sxs = []
            for (h, pb, j, out_ps), stp in zip(heads, sts):
                sx = cx.sxp.tile([P, SC], DT_MM, tag="sx")
                nc.scalar.activation(sx[:, :w], stp[:, :w], EXP, scale=SCALE)
                sxs.append(sx)
            st["sx"][i] = sxs
        return run

    def make_pv(i, kb, jj):
        def run():
            off = 0 if jj is None else jj * P
            w = SC - off
            sxs = st["sx"].pop(i)
            for (h, pb, j, out_ps), sx in zip(st["heads"], sxs):
                nc.tensor.matmul(
                    out_ps[:, off:], cx.V[:, kb, h, :], sx[:, :w],
                    start=(i == 0), stop=(i == nblk - 1))
        return run

    def drain():
        st["raws"] = []
        for h, pb, j, out_ps in st["heads"]:
            raw = cx.nrmraw.tile([65, SC], F32, tag="raw")
            nc.vector.tensor_copy(raw, out_ps)
            st["raws"].append(raw)

    def norm():
        for (h, pb, j, out_ps), raw in zip(st["heads"], st["raws"]):
            # Single-partition reciprocal blocks the DVE FIFO for ~us;
            # DMA-scatter the sums across 128 partitions first.
            rsh = cx.nrmbc.tile([P, SC // P], F32, tag="rsh")
            nc.sync.dma_start(rsh, raw[64:65, :])
            nc.vector.reciprocal(rsh, rsh)
            rdram = cx.drp.tile([1, SC], F32, tag="rdram")
            nc.sync.dma_start(rdram, rsh)
            rb = cx.nrmbc.tile([64, SC], F32, tag="rb")
            nc.sync.dma_start(rb, rdram.to_broadcast([64, SC]))
            stg = cx.nrmbc.tile([64, SC], BF16, tag="stg")
            nc.vector.tensor_mul(stg, raw[0:64, :], rb)
            nc.sync.dma_start(cx.attnTc[q][pb:pb + 64, j, :], stg)

    scores_t = [make_scores(i, kb, jj) for i, (kb, jj) in enumerate(blocks)]
    pv_t = [make_pv(i, kb, jj) for i, (kb, jj) in enumerate(blocks)]
    thunks = [setup, scores_t[0]]
    for i in range(1, nblk):
        thunks.append(scores_t[i])
        thunks.append(pv_t[i - 1])
    thunks.append(pv_t[nblk - 1])
    thunks += [drain, norm]
    return thunks


def _attention_chunk_thunks(nc, cx, q):
    out = []
    for hp in range(HG // 2):
        out += _attention_pair_thunks(nc, cx, 2 * hp, 2 * hp + 1, q)
    return out


def _proj_chunk_thunks(nc, cx, q, out_d):
    """Proj for the s-blocks of chunk q (bf16); two thunks per s-block."""
    thunks = []
    for sb in range(SC // P):
        sblk = q * (SC // P) + sb

        def make_half(hf, sblk=sblk, sb=sb):
            def run():
                og = cx.ogp.tile([P, D // 2], F32, tag="og")
                ps = cx.ps1.tile([P, D // 2], F32, tag="qkps")
                n0 = hf * (D // 2)
                for j in range(FBN):
                    nc.tensor.matmul(
                        ps,
                        cx.attnTc[q][:, j, sb * P:(sb + 1) * P],
                        cx.wp_sb[:, j, n0:n0 + D // 2],
                        start=(j == 0), stop=(j == FBN - 1))
                nc.any.tensor_copy(og, ps)
                nc.sync.dma_start(
                    out_d.ap()[sblk * P:(sblk + 1) * P, n0:n0 + D // 2], og)
            return run

        thunks.append(make_half(0))
        thunks.append(make_half(1))
    return thunks


def _body(tc, x_d, wq_d, wk_d, wv_d, wp_d, bq_d, bk_d, out_d):
    nc = tc.nc
    cx = _Ctx()
    XC = 256                  # QKV s-chunk width
    NXC = S // XC             # 8
    with (
        tc.tile_pool(name="persist", bufs=1) as persist,
        tc.tile_pool(name="ph1", bufs=1) as ph1,
        tc.tile_pool(name="xin", bufs=3) as xinp,
        tc.tile_pool(name="xtp", bufs=2) as xtp,
        tc.tile_pool(name="xtp8", bufs=2) as xtp8,
        tc.tile_pool(name="qtc", bufs=2) as qtc,
        tc.tile_pool(name="atc", bufs=2) as atc,
        tc.tile_pool(name="sxp", bufs=4) as sxp,
        tc.tile_pool(name="nrmraw", bufs=3) as nrmraw,
        tc.tile_pool(name="nrmbc", bufs=2) as nrmbc,
        tc.tile_pool(name="ogp", bufs=2) as ogp,
        # PSUM banks: qkps 2 + (pt+stps shared) 4 + outps 2 = 8
        tc.tile_pool(name="ps1", bufs=2, space="PSUM") as ps1,
        tc.tile_pool(name="psst", bufs=4, space="PSUM") as psst,
        tc.tile_pool(name="psout", bufs=2, space="PSUM") as psout,
        tc.tile_pool(name="drp", bufs=8, space="DRAM") as drp,
    ):
        cx.sxp, cx.nrmraw, cx.nrmbc, cx.ogp = sxp, nrmraw, nrmbc, ogp
        cx.psst, cx.psout, cx.drp, cx.ps1 = psst, psout, drp, ps1

        ident = persist.tile([P, P], F32)
        make_identity(nc, ident)
        ident16 = persist.tile([P, P], BF16)
        nc.vector.tensor_copy(ident16, ident)
        for _ in range(12):
            wp_ps = ps1.tile([P, P], F32, tag="qkps")
            nc.tensor.matmul(wp_ps, ident, ident, start=True, stop=True)
        cx.addmask = persist.tile([P, P], F32)
        make_lower_triangular(nc, cx.addmask, val=MASKVAL, diag=False)
        bq_sb = persist.tile([P, FBN], F32)
        bk_sb = persist.tile([P, FBN], F32)
        nc.sync.dma_start(bq_sb, bq_d.ap().rearrange("(j p) -> p j", p=P))
        nc.sync.dma_start(bk_sb, bk_d.ap().rearrange("(j p) -> p j", p=P))

        cx.KT = persist.tile([P, FBN, S], BF16)
        cx.V = persist.tile([P, NKB, HG, HD + 1], DT_MM)
        ones_col = persist.tile([P, 1], F32)
        nc.vector.memset(ones_col, 1.0)
        nc.vector.tensor_copy(cx.V[:, :, :, HD],
                              ones_col.to_broadcast([P, NKB, HG]))
        cx.wp_sb = persist.tile([P, FBN, D], BF16)
        cx.QTc = [qtc.tile([P, FBN, SC], BF16, tag="qtc", name=f"qtc{q}")
                  for q in range(NQ)]
        cx.attnTc = [atc.tile([P, FBN, SC], BF16, tag="atc",
                              name=f"atc{q}") for q in range(NQ)]

        wq_sb = ph1.tile([P, DB, FG], FP8)
        wk_sb = ph1.tile([P, DB, FG], FP8)
        wv_sb = ph1.tile([P, DB, FG], BF16)

        def transpose_chunk(xc, xt, xt8):
            thunks = []
            for sb in range(XC // P):
                s0 = xc * XC + sb * P
                for dh in range(2):
                    xin = xinp.tile([P, D // 2], BF16, tag="xin")
                    nc.sync.dma_start(
                        xin, x_d.ap()[s0:s0 + P,
                                      dh * (D // 2):(dh + 1) * (D // 2)])
                    if xc == 0:
                        # Paced pre-warm: junk full-array matmuls keyed to
                        # the arriving input DMAs keep the PE busy through
                        # the initial load window.
                        wp_ps = ps1.tile([P, P], F32, tag="qkps")
                        nc.tensor.matmul(wp_ps, ident16, xin[:, 0:P],
                                         start=True, stop=True)
                    for db4 in range(DB // 2):
                        db = dh * (DB // 2) + db4
                        def t(sb=sb, db=db, db4=db4, xin=xin, xc=xc):
                            pt = cx.psst.tile([P, P], BF16, tag="stps")
                            nc.tensor.transpose(
                                pt, xin[:, db4 * P:(db4 + 1) * P], ident16)
                            nc.vector.tensor_copy(
                                xt[:, db, sb * P:(sb + 1) * P], pt)
                            eng = nc.vector if xc < 2 else nc.gpsimd
                            eng.tensor_copy(
                                xt8[:, db, sb * P:(sb + 1) * P],
                                xt[:, db, sb * P:(sb + 1) * P])
                        thunks.append(t)
            return thunks

        xts = [xtp.tile([P, DB, XC], BF16, tag="xt", name=f"xt{xc}")
               for xc in range(NXC)]
        xt8s = [xtp8.tile([P, DB, XC], FP8, tag="xt8", name=f"xt8{xc}")
                for xc in range(NXC)]

        bg = []          # attention/proj thunks dripped between QKV groups
        tr = []          # transpose thunks for the next chunk

        def drip(ntr, nbg):
            for _ in range(ntr):
                if tr:
                    tr.pop(0)()
            for _ in range(nbg):
                if bg:
                    bg.pop(0)()

        for t in transpose_chunk(0, xts[0], xt8s[0]):
            t()
        for w_sb, w_d in ((wq_sb, wq_d), (wk_sb, wk_d), (wv_sb, wv_d)):
            wr = w_d.ap().rearrange("(db p) f -> db p f", p=P)
            for db in range(DB):
                nc.sync.dma_start(w_sb[:, db], wr[db])
        nc.sync.dma_start(
            cx.wp_sb, wp_d.ap().rearrange("(j p) n -> p j n", p=P))

        for xc in range(NXC):
            xt, xt8 = xts[xc], xt8s[xc]
            q, half = divmod(xc, 2)
            if xc + 1 < NXC:
                tr += transpose_chunk(xc + 1, xts[xc + 1], xt8s[xc + 1])
            if half == 0:
                if q >= 1:
                    bg += _attention_chunk_thunks(nc, cx, q - 1)
                if q >= 2:
                    bg += _proj_chunk_thunks(nc, cx, q - 2, out_d)
            per = (len(bg) + 9) // 10

            # Q and K -> transposed feature-major layout via fp8
            # DoubleRow; 1/WSCALE and bias folded into the copy out.
            for w_sb, Tc, b_sb in ((wq_sb, cx.QTc, bq_sb),
                                   (wk_sb, None, bk_sb)):
                for fb in range(FBN):
                    ps = ps1.tile([P, XC], F32, tag="qkps")
                    for j in range(DB // 2):
                        nc.tensor.matmul(
                            ps,
                            w_sb[:, 2 * j:2 * j + 2,
                                 fb * P:(fb + 1) * P],
                            xt8[:, 2 * j:2 * j + 2, :],
                            start=(j == 0), stop=(j == DB // 2 - 1),
                            perf_mode=DR)
                        drip(1, 0)
                    if Tc is not None:
                        dst = Tc[q][:, fb, half * XC:(half + 1) * XC]
                    else:
                        dst = cx.KT[:, fb, xc * XC:(xc + 1) * XC]
                    nc.vector.tensor_scalar(dst, ps, WINV,
                                            b_sb[:, fb:fb + 1], MULT, ADD)
                    drip(0, per)
            # V -> natural [s, feat] layout, bf16 (no bias: host-folded).
            for sb in range(XC // P):
                kb = xc * (XC // P) + sb
                ps = ps1.tile([P, FG], F32, tag="qkps")
                for db in range(DB):
                    nc.tensor.matmul(
                        ps,
                        xt[:, db, sb * P:(sb + 1) * P],
                        wv_sb[:, db, :],
                        start=(db == 0), stop=(db == DB - 1))
                    drip(1 if db % 2 else 0, 0)
                nc.vector.tensor_copy(
                    cx.V[:, kb, :, 0:HD],
                    ps.rearrange("p (h c) -> p h c", h=HG))
                drip(0, per)
            while tr:
                tr.pop(0)()

        # Tail: attention(3) interleaved with proj(2), then proj(3).
        tail_att = _attention_chunk_thunks(nc, cx, NQ - 1)
        tail_proj = _proj_chunk_thunks(nc, cx, NQ - 2, out_d)
        k = max(1, len(tail_att) // max(1, len(tail_proj)))
        while tail_att or tail_proj:
            for _ in range(k):
                if tail_att:
                    tail_att.pop(0)()
            if tail_proj:
                tail_proj.pop(0)()
        while bg:
            bg.pop(0)()
        for t in _proj_chunk_thunks(nc, cx, NQ - 1, out_d):
            t()


def build_nc():
    nc = bacc.Bacc("TRN2", target_bir_lowering=False)
    x_d = nc.dram_tensor("x", [S, D], BF16, kind="ExternalInput")
    wq_d = nc.dram_tensor("wq", [D, FG], FP8, kind="ExternalInput")
    wk_d = nc.dram_tensor("wk", [D, FG], FP8, kind="ExternalInput")
    wv_d = nc.dram_tensor("wv", [D, FG], BF16, kind="ExternalInput")
    wp_d = nc.dram_tensor("wp", [FG, D], BF16, kind="ExternalInput")
    bq_d = nc.dram_tensor("bq", [FG], F32, kind="ExternalInput")
    bk_d = nc.dram_tensor("bk", [FG], F32, kind="ExternalInput")
    out_d = nc.dram_tensor("out", [S, D], F32, kind="ExternalOutput")
    with tile.TileContext(nc) as tc:
        _body(tc, x_d, wq_d, wk_d, wv_d, wp_d, bq_d, bk_d, out_d)
    nc.compile()
    return nc


_NC = None


def _get_nc():
    global _NC
    if _NC is None:
        _NC = build_nc()
    return _NC


def make_in_maps(hs, w, bvec, pw):
    import ml_dtypes
    fp8 = ml_dtypes.float8_e4m3
    bf16 = ml_dtypes.bfloat16
    in_maps = []
    for core in range(NCORES):
        b, g = divmod(core, 2)
        lo, hi = g * FG, (g + 1) * FG
        in_maps.append({
            "x": np.ascontiguousarray(hs[b]).astype(bf16),
            "wq": np.ascontiguousarray(
                w[:, lo:hi] * WSCALE).astype(fp8),
            "wk": np.ascontiguousarray(
                w[:, D + lo:D + hi] * WSCALE).astype(fp8),
            "wv": np.ascontiguousarray(
                w[:, 2 * D + lo:2 * D + hi]).astype(bf16),
            "wp": np.ascontiguousarray(pw[lo:hi, :]).astype(bf16),
            "bq": np.ascontiguousarray(bvec[lo:hi]),
            "bk": np.ascontiguousarray(bvec[D + lo:D + hi]),
        })
    return in_maps


def combine(parts, bvec, pw, pb):
    bv = bvec[2 * D:3 * D].astype(np.float64)
    corr = (bv @ pw.astype(np.float64) + pb.astype(np.float64)).astype(
        np.float32)
    out = np.empty((B, S, D), np.float32)
    for b in range(B):
        out[b] = parts[2 * b] + parts[2 * b + 1] + corr
    return out


def kernel(hidden_states, c_attn_w, c_attn_b, c_proj_w, c_proj_b,
           **run_kwargs):
    hs = np.asarray(hidden_states, dtype=np.float32)
    w = np.asarray(c_attn_w, dtype=np.float32)
    bvec = np.asarray(c_attn_b, dtype=np.float32)
    pw = np.asarray(c_proj_w, dtype=np.float32)
    pb = np.asarray(c_proj_b, dtype=np.float32)
    nc = _get_nc()
    res = run_bass_kernel_spmd(nc, make_in_maps(hs, w, bvec, pw),
                               core_ids=list(range(NCORES)), **run_kwargs)
    parts = [res.results[i]["out"] for i in range(NCORES)]
    out = combine(parts, bvec, pw, pb)
    if run_kwargs:
        return out, res
    return out
